# revision 2
# baseline (speedup 1.0000x reference)
"""Trainium2 Bass kernel for fused multi-head causal attention (GPT-2 style).

Full-input contract: kernel(**inputs) takes the complete tensors and returns
the complete output. Internally: data-parallel over the batch dim (B=8) across
8 NeuronCores; each core runs the whole attention block for one batch element.

Per-core dataflow (S=512, D=1024, H=16, dh=64), everything fp32r on the PE:

  x^T [D,S]  (host-transposed, resident in SBUF)
  V:   psum[s,n]   = x^T[:,s].T @ W1v            (normal layout, into [V|1]/[1|V] blocks)
  QK:  psum[n,s]   = W1[:,n].T @ x^T             (n-tiles on partitions -> Q^T,K^T)
  S^T: psum[sk,sq] = K_h^T[:,sk].T @ Q_h^T       (scores transposed, causal-trimmed)
  P^T  = exp(S^T/8 + pad_bias); tril mask on diag block zeroes the future
  A^T: psum = [V_h|1].T @ P^T                    (64 attn rows + 64 denominator rows)
  A^T_norm = psum_attn * recip(psum_denom)       -> A^T tiles [n, s]
  out: psum[s,e]   = A^T[:,s].T @ W2             + b2

Heads are processed in pairs (even head on partitions 0:64, odd on 64:128 of
their shared n-tile) with score matmuls emitted adjacently so the two K=64
matmuls can overlap in distinct PE row-groups.
"""

import sys

if "/opt/trn_rl_repo" not in sys.path:
    sys.path.insert(0, "/opt/trn_rl_repo")

import numpy as np

import concourse.bass as bass
import concourse.mybir as mybir
import concourse.tile as tile
from concourse import bacc
from concourse.bass_utils import run_bass_kernel_spmd
from concourse.masks import make_identity, make_upper_triangular

F32 = mybir.dt.float32
F32R = mybir.dt.float32r
BF16 = mybir.dt.bfloat16

B, S, D = 8, 512, 1024
H = 16
DH = D // H          # 64
NT_S = S // 128      # 4 s-tiles
ND = D // 128        # 8 d-tiles
N_CORES = 8
SCALE = 1.0 / 8.0    # 1/sqrt(head_dim)

_CACHED = {}
VARIANT = {"mask": "pe"}  # "dve" | "pe"


def _dram_ap(t, offset, dims):
    """Raw strided DRAM access pattern ([step, count] pairs, elements)."""
    return bass.AP(tensor=t[...].tensor if hasattr(t, "shape") else t.tensor,
                   offset=offset, ap=dims)


def _emit_body(nc, tc, pools, dram, consts, phases=("v", "qk", "attn", "cproj")):
    (xt_p, w1v_p, w1qk_p, qkt_p, vsb_p, pt_p, recip_p, at_p, w2sb_p, out_p,
     mmps_p, scps_p, avps_p) = pools
    xt, msk, w1, w1qk_pk, b1, w2, b2, out = dram
    tril01, idn, trilneg, pad_bias, bqk, bv, bo, ones_f32 = consts

    # ---------------- resident x^T : one strided DMA ----------------
    xt_sb = xt_p.tile([128, ND, S], F32R, tag="xt")
    nc.sync.dma_start(
        xt_sb[:], _dram_ap(xt, 0, [[S, 128], [128 * S, ND], [1, S]]).bitcast(F32R)
    )

    # ---------------- W1 V-columns + W2, one strided DMA each (ACT queue) ---
    w1v = w1v_p.tile([128, ND, D], F32R, tag="w1v")
    nc.scalar.dma_start(
        w1v[:], _dram_ap(w1, 2 * D, [[3 * D, 128], [128 * 3 * D, ND], [1, D]]).bitcast(F32R)
    )
    w2sb = w2sb_p.tile([128, ND, D], F32R, tag="w2sb")
    nc.scalar.dma_start(
        w2sb[:], _dram_ap(w2, 0, [[D, 128], [128 * D, ND], [1, D]]).bitcast(F32R)
    )

    # ---------------- V projection into [V|1]/[1|V] head blocks ----------
    va_sb = vsb_p.tile([128, NT_S, H, 128], F32R, tag="va")
    do_v = "v" in phases
    do_qk = "qk" in phases
    do_attn = "attn" in phases
    do_cproj = "cproj" in phases
    # ones half-blocks: for pair p, flat cols [256p+64, 256p+192) (gpsimd copy)
    for t in range(NT_S) if do_v else []:
        for p in range(H // 2):
            vv = va_sb[:, t, 2 * p : 2 * p + 2, :]
            ones_dst = bass.AP(tensor=vv.tensor, offset=vv.offset + 64,
                               ap=[vv.ap[0], [1, 128]])
            nc.gpsimd.tensor_copy(ones_dst, ones_f32[:])

    for t in range(NT_S) if do_v else []:
        for c in range(2):
            ps = mmps_p.tile([128, 512], F32, tag="mmps")
            for d in range(ND):
                nc.tensor.matmul(
                    ps[:],
                    xt_sb[:, d, t * 128 : (t + 1) * 128],
                    w1v[:, d, c * 512 : (c + 1) * 512],
                    start=(d == 0), stop=(d == ND - 1),
                )
            for hh in range(8):
                h = 8 * c + hh
                dst = (va_sb[:, t, h, 0:64] if h % 2 == 0
                       else va_sb[:, t, h, 64:128])
                nc.vector.tensor_tensor(
                    out=dst, in0=ps[:, hh * 64 : hh * 64 + 64],
                    in1=bv[:, c * 512 + hh * 64 : c * 512 + hh * 64 + 64],
                    op=mybir.AluOpType.add,
                )

    # ------------- QK projection pass i -> attention for heads 2i, 2i+1 ----
    # Software pipeline: per iteration, emit pair i's score matmuls, then pass
    # i+1's QK projection (independent PE work that hides pair i's ACT exps),
    # then pair i's A^T matmuls.
    at_sb = at_p.tile([128, ND, S], F32R, tag="at")

    def emit_qk_pass(i):
        # w1 columns {128i..128i+128} (Q) and {D+128i..} (K) for all 8 d-tiles
        wqk = w1qk_p.tile([128, ND, 2, 128], F32R, tag="w1qk")
        if VARIANT.get("wqk", "packed") == "packed":
            nc.sync.dma_start(wqk[:], w1qk_pk[i].bitcast(F32R))
        else:
            for j in range(2):
                nc.sync.dma_start(
                    wqk[:, :, j, :],
                    _dram_ap(w1, 128 * i + j * D,
                             [[3 * D, 128], [128 * 3 * D, ND], [1, 128]]).bitcast(F32R),
                )
        if not do_qk:
            return None, None
        psq = mmps_p.tile([128, 512], F32, tag="mmps")
        psk = mmps_p.tile([128, 512], F32, tag="mmps")
        for d in range(ND):
            nc.tensor.matmul(psq[:], wqk[:, d, 0, :], xt_sb[:, d, :],
                             start=(d == 0), stop=(d == ND - 1))
            nc.tensor.matmul(psk[:], wqk[:, d, 1, :], xt_sb[:, d, :],
                             start=(d == 0), stop=(d == ND - 1))
        qt = qkt_p.tile([128, S], F32R, tag="qkt")
        kt = qkt_p.tile([128, S], F32R, tag="qkt")
        nc.vector.tensor_scalar(out=qt[:], in0=psq[:], scalar1=bqk[:, i : i + 1],
                                scalar2=None, op0=mybir.AluOpType.add)
        nc.vector.tensor_scalar(out=kt[:], in0=psk[:], scalar1=bqk[:, ND + i : ND + i + 1],
                                scalar2=None, op0=mybir.AluOpType.add)
        return qt, kt

    qt, kt = emit_qk_pass(0)
    for i in range(ND):
        if not do_attn:
            if i + 1 < ND:
                qt, kt = emit_qk_pass(i + 1)
            continue
        h_e, h_o = 2 * i, 2 * i + 1
        av_e = avps_p.tile([128, 512], F32, tag="avps")
        av_o = avps_p.tile([128, 512], F32, tag="avps")
        # --- scores for all sk (feeds ACT early) ---
        pts = []
        for sk in range(NT_S):
            w = S - sk * 128
            # both heads' scores in one 2-bank tile -> single paired exp
            sc = scps_p.tile([128, 2, 512], F32, tag="scps")
            mask_on_pe = VARIANT.get("mask") == "pe"
            nc.tensor.matmul(sc[:, 0, 0:w], kt[0:64, sk * 128 : (sk + 1) * 128],
                             qt[0:64, sk * 128 : S], start=True, stop=not mask_on_pe)
            nc.tensor.matmul(sc[:, 1, 0:w], kt[64:128, sk * 128 : (sk + 1) * 128],
                             qt[64:128, sk * 128 : S], start=True, stop=not mask_on_pe)
            if mask_on_pe:
                nc.tensor.matmul(sc[:, 0, 0:w], idn[:], trilneg[:, 0:w],
                                 start=False, stop=True)
                nc.tensor.matmul(sc[:, 1, 0:w], idn[:], trilneg[:, 0:w],
                                 start=False, stop=True)
            pt = pt_p.tile([128, 2, 512], F32R, tag="pt")
            nc.scalar.activation(pt[:, :, 0:w], sc[:, :, 0:w],
                                 mybir.ActivationFunctionType.Exp,
                                 bias=pad_bias[:, sk : sk + 1], scale=SCALE)
            if not mask_on_pe:
                # zero strictly-future entries of the diagonal block (per head)
                for j in range(2):
                    nc.vector.tensor_tensor(out=pt[:, j, 0:128], in0=pt[:, j, 0:128],
                                            in1=tril01[:], op=mybir.AluOpType.mult)
            pts.append(pt)
        # --- next pass's QK projection: PE work independent of the exps ---
        if i + 1 < ND:
            qt_n, kt_n = emit_qk_pass(i + 1)
        # --- A^T accumulation (exps are done or nearly done by now) ---
        for sk in range(NT_S):
            w = S - sk * 128
            for j, (h, av) in enumerate(((h_e, av_e), (h_o, av_o))):
                nc.tensor.matmul(av[:, sk * 128 : S], va_sb[:, sk, h, :],
                                 pts[sk][:, j, 0:w],
                                 start=(sk == 0), stop=(sk == NT_S - 1))
        # normalize: attn rows * recip(denom rows)
        rc = recip_p.tile([128, 512], F32, tag="recip")
        nc.vector.reciprocal(rc[0:64, :], av_e[64:128, :])
        nc.vector.tensor_tensor(out=at_sb[0:64, i, :], in0=av_e[0:64, :],
                                in1=rc[0:64, :], op=mybir.AluOpType.mult)
        nc.vector.reciprocal(rc[64:128, :], av_o[0:64, :])
        nc.vector.tensor_tensor(out=at_sb[64:128, i, :], in0=av_o[64:128, :],
                                in1=rc[64:128, :], op=mybir.AluOpType.mult)
        if i + 1 < ND:
            qt, kt = qt_n, kt_n

    # ---------------- c_proj ----------------
    for t in range(NT_S) if do_cproj else []:
        for c in range(2):
            ps = mmps_p.tile([128, 512], F32, tag="mmps")
            for d in range(ND):
                nc.tensor.matmul(
                    ps[:],
                    at_sb[:, d, t * 128 : (t + 1) * 128],
                    w2sb[:, d, c * 512 : (c + 1) * 512],
                    start=(d == 0), stop=(d == ND - 1),
                )
            ob = out_p.tile([128, 512], F32, tag="outsb")
            nc.vector.tensor_tensor(out=ob[:], in0=ps[:],
                                    in1=bo[:, c * 512 : (c + 1) * 512],
                                    op=mybir.AluOpType.add)
            nc.scalar.dma_start(
                out[t * 128 : (t + 1) * 128, c * 512 : (c + 1) * 512], ob[:]
            )


def _build_nc(repeats=1, loop_n=None, phases=("v", "qk", "attn", "cproj")):
    nc = bacc.Bacc("TRN2", target_bir_lowering=False, debug=False)

    xt = nc.dram_tensor("xt", [D, S], F32, kind="ExternalInput")          # x[b].T
    msk = nc.dram_tensor("msk", [S], F32, kind="ExternalInput")
    w1 = nc.dram_tensor("w1", [D, 3 * D], F32, kind="ExternalInput")      # c_attn_w
    w1qk_pk = nc.dram_tensor("w1qk_pk", [ND, 128, ND * 2 * 128], F32,
                             kind="ExternalInput")  # host-packed QK slices
    b1 = nc.dram_tensor("b1", [3 * D], F32, kind="ExternalInput")
    w2 = nc.dram_tensor("w2", [D, D], F32, kind="ExternalInput")          # c_proj_w
    b2 = nc.dram_tensor("b2", [D], F32, kind="ExternalInput")
    out = nc.dram_tensor("out", [S, D], F32, kind="ExternalOutput")
    dram = (xt, msk, w1, w1qk_pk, b1, w2, b2, out)

    with tile.TileContext(nc) as tc:
        with (
            tc.tile_pool(name="const", bufs=1) as const_p,
            tc.tile_pool(name="xt", bufs=1) as xt_p,
            tc.tile_pool(name="w1v", bufs=1) as w1v_p,
            tc.tile_pool(name="w1qk", bufs=3) as w1qk_p,
            tc.tile_pool(name="qkt", bufs=4) as qkt_p,
            tc.tile_pool(name="vsb", bufs=1) as vsb_p,
            tc.tile_pool(name="pt", bufs=5) as pt_p,
            tc.tile_pool(name="recip", bufs=2) as recip_p,
            tc.tile_pool(name="at", bufs=1) as at_p,
            tc.tile_pool(name="w2sb", bufs=1) as w2sb_p,
            tc.tile_pool(name="outsb", bufs=2) as out_p,
            tc.tile_pool(name="mmps", bufs=2, space="PSUM") as mmps_p,
            tc.tile_pool(name="scps", bufs=2, space="PSUM") as scps_p,
            tc.tile_pool(name="avps", bufs=2, space="PSUM") as avps_p,
        ):
            # ---- constants (once) ----
            # multiplicative causal mask for the diagonal block:
            # keep[k, q] = 1 if q >= k else 0
            tril01 = const_p.tile([128, 128], F32)
            make_upper_triangular(nc, tril01[:], val=1.0, diag=True)
            # additive variant consts for mask-on-PE
            idn = const_p.tile([128, 128], BF16)
            make_identity(nc, idn[:])
            trilneg = const_p.tile([128, 512], BF16)
            nc.gpsimd.memset(trilneg[:], 0.0)
            nc.gpsimd.affine_select(
                out=trilneg[:], in_=trilneg[:],
                compare_op=mybir.AluOpType.is_ge, fill=-1e9, base=0,
                pattern=[[1, 512]], channel_multiplier=-1,
            )
            ones_f32 = const_p.tile([128, 128], F32)
            nc.gpsimd.memset(ones_f32[:], 1.0)

            msk_sb = const_p.tile([128, NT_S], F32)
            nc.sync.dma_start(msk_sb[:], _dram_ap(msk, 0, [[1, 128], [128, NT_S]]))
            pad_bias = const_p.tile([128, NT_S], F32)
            nc.vector.tensor_scalar(
                out=pad_bias[:], in0=msk_sb[:], scalar1=1.0, scalar2=1e9,
                op0=mybir.AluOpType.subtract, op1=mybir.AluOpType.mult,
            )

            bqk = const_p.tile([128, 2 * ND], F32)
            nc.scalar.dma_start(bqk[:], _dram_ap(b1, 0, [[1, 128], [128, 2 * ND]]))
            bv = const_p.tile([128, D], F32)
            nc.gpsimd.dma_start(out=bv[:], in_=b1[None, 2 * D : 3 * D].to_broadcast([128, D]))
            bo = const_p.tile([128, D], F32)
            nc.gpsimd.dma_start(out=bo[:], in_=b2[None, :].to_broadcast([128, D]))

            consts = (tril01, idn, trilneg, pad_bias, bqk, bv, bo, ones_f32)
            pools = (xt_p, w1v_p, w1qk_p, qkt_p, vsb_p, pt_p, recip_p, at_p,
                     w2sb_p, out_p, mmps_p, scps_p, avps_p)
            if loop_n is not None:
                with tc.For_i(0, loop_n, 1):
                    _emit_body(nc, tc, pools, dram, consts, phases)
            else:
                for _ in range(repeats):
                    _emit_body(nc, tc, pools, dram, consts, phases)

    nc.compile()
    return nc


def _get_nc(repeats=1, loop_n=None, phases=("v", "qk", "attn", "cproj")):
    key = ("nc", repeats, loop_n, tuple(phases), tuple(sorted(VARIANT.items())))
    if key not in _CACHED:
        _CACHED[key] = _build_nc(repeats, loop_n, phases)
    return _CACHED[key]


def _pack_w1qk(w1):
    # pack per-pass QK weight slices: pass i needs w1[:, 128i:128i+128] (Q)
    # and w1[:, D+128i:D+128i+128] (K) for each of the 8 d-tiles, laid out
    # [pass, partition, d, {q,k}, col] so each pass is one contiguous DMA
    w1 = np.asarray(w1, dtype=np.float32)
    w1r = w1.reshape(ND, 128, 3 * D)
    qs = w1r[:, :, :D].reshape(ND, 128, ND, 128)       # [d, p, i, c]
    ks = w1r[:, :, D:2 * D].reshape(ND, 128, ND, 128)
    pk = np.stack([qs, ks], axis=3)                    # [d, p, i, {q,k}, c]
    return np.ascontiguousarray(pk.transpose(2, 1, 0, 3, 4).reshape(
        ND, 128, ND * 2 * 128))


def _trace_setup(inputs):
    """Build (nc, in_maps) exactly as kernel() would — for test.py tracing."""
    x = np.asarray(inputs["x"], dtype=np.float32)
    mask = np.asarray(inputs["mask"], dtype=np.float32)
    w1 = np.ascontiguousarray(np.asarray(inputs["c_attn_w"], dtype=np.float32))
    b1 = np.ascontiguousarray(np.asarray(inputs["c_attn_b"], dtype=np.float32))
    w2 = np.ascontiguousarray(np.asarray(inputs["c_proj_w"], dtype=np.float32))
    b2 = np.ascontiguousarray(np.asarray(inputs["c_proj_b"], dtype=np.float32))
    nc = _get_nc()
    w1qk_pk = _pack_w1qk(w1)
    in_maps = []
    for b in range(N_CORES):
        in_maps.append({
            "xt": np.ascontiguousarray(x[b].T),
            "msk": np.ascontiguousarray(mask[b]),
            "w1": w1, "w1qk_pk": w1qk_pk, "b1": b1, "w2": w2, "b2": b2,
        })
    return nc, in_maps


def kernel(x, mask, c_attn_w, c_attn_b, c_proj_w, c_proj_b):
    x = np.asarray(x, dtype=np.float32)
    mask = np.asarray(mask, dtype=np.float32)
    w1 = np.ascontiguousarray(np.asarray(c_attn_w, dtype=np.float32))
    b1 = np.ascontiguousarray(np.asarray(c_attn_b, dtype=np.float32))
    w2 = np.ascontiguousarray(np.asarray(c_proj_w, dtype=np.float32))
    b2 = np.ascontiguousarray(np.asarray(c_proj_b, dtype=np.float32))

    nc = _get_nc()
    w1qk_pk = _pack_w1qk(w1)
    in_maps = []
    for b in range(N_CORES):
        in_maps.append({
            "xt": np.ascontiguousarray(x[b].T),
            "msk": np.ascontiguousarray(mask[b]),
            "w1": w1, "w1qk_pk": w1qk_pk, "b1": b1, "w2": w2, "b2": b2,
        })
    res = run_bass_kernel_spmd(nc, in_maps, list(range(N_CORES)))
    return np.stack([res.results[b]["out"] for b in range(N_CORES)], axis=0)



# revision 6
# speedup vs baseline: 1.3592x; 1.3592x over previous
"""Trainium2 Bass kernel for fused multi-head causal attention (GPT-2 style).

Full-input contract: kernel(**inputs) takes the complete tensors and returns
the complete output. Internally: data-parallel over the batch dim (B=8) across
8 NeuronCores; each core runs the whole attention block for one batch element.

Per-core dataflow (S=512, D=1024, H=16, dh=64). All matmul operands are bf16
(PSUM accumulation stays fp32); rel-err tolerance is 2e-2 so bf16 rounding
(~0.5%) is safe and doubles effective PE/DMA throughput vs fp32:

  x^T [128, d, S]   host-packed bf16, resident in SBUF (2 DMA queues)
  V:   psum[s,n]   = x^T[:,s].T @ W1v        -> [V|1]/[1|V] head blocks in SBUF
  QK:  psum[n,s]   = W1qk[:,n].T @ x^T       -> Q^T,K^T   (bias on ACT engine)
  S^T: psum[sk,sq] = K_h^T[:,sk].T @ Q_h^T   (scores transposed, causal-trimmed)
  P^T  = exp(S^T/8 + pad_bias) on ACT; tril multiply on DVE zeroes the future
  A^T: psum = [V_h|1].T @ P^T                (64 attn rows + 64 denominator rows)
  A^T_norm = psum_attn * recip_approx(psum_denom)  -> A^T bf16 tiles [n, s]
  out: psum[s,e]   = A^T[:,s].T @ W2         + b2 (fp32 out)

Schedule is ordered by DMA arrival: QK pass 0 (needs x^T + 0.5MB of weights)
runs first, then the V projection c-half 0; head pairs are software-pipelined
with the next QK pass (or V c-half 1) emitted between a pair's score matmuls
and its A^T matmuls so the PE never waits on the ACT-engine exps.
"""

import sys

if "/opt/trn_rl_repo" not in sys.path:
    sys.path.insert(0, "/opt/trn_rl_repo")

import numpy as np

import concourse.bass as bass
import concourse.mybir as mybir
import concourse.tile as tile
from concourse import bacc
from concourse.bass_utils import run_bass_kernel_spmd
from concourse.masks import make_upper_triangular

F32 = mybir.dt.float32
BF16 = mybir.dt.bfloat16
NPBF16 = mybir.dt.np(BF16)

B, S, D = 8, 512, 1024
H = 16
DH = D // H          # 64
NT_S = S // 128      # 4 s-tiles
ND = D // 128        # 8 d-tiles
N_CORES = 8
SCALE = 1.0 / 8.0    # 1/sqrt(head_dim)

_CACHED = {}


def _dram_ap(t, offset, dims):
    """Raw strided DRAM access pattern ([step, count] pairs, elements)."""
    return bass.AP(tensor=t[...].tensor, offset=offset, ap=dims)


def _build_nc():
    nc = bacc.Bacc("TRN2", target_bir_lowering=False, debug=False)

    Exp = mybir.ActivationFunctionType.Exp
    Ident = mybir.ActivationFunctionType.Identity
    ADD = mybir.AluOpType.add
    MUL = mybir.AluOpType.mult

    # host-packed bf16 inputs (all [128 partitions, ...] contiguous)
    xt_lo = nc.dram_tensor("xt_lo", [128, 4, S], BF16, kind="ExternalInput")
    xt_hi = nc.dram_tensor("xt_hi", [128, 4, S], BF16, kind="ExternalInput")
    w1v0_d = nc.dram_tensor("w1v0", [128, ND, 512], BF16, kind="ExternalInput")
    w1v1_d = nc.dram_tensor("w1v1", [128, ND, 512], BF16, kind="ExternalInput")
    wqk_pk = nc.dram_tensor("wqk_pk", [ND, 128, ND, 2, 128], BF16,
                            kind="ExternalInput")
    w2_d = nc.dram_tensor("w2pk", [128, ND, D], BF16, kind="ExternalInput")
    msk = nc.dram_tensor("msk", [S], F32, kind="ExternalInput")
    b1 = nc.dram_tensor("b1", [3 * D], F32, kind="ExternalInput")
    b2 = nc.dram_tensor("b2", [D], F32, kind="ExternalInput")
    out = nc.dram_tensor("out", [S, D], F32, kind="ExternalOutput")

    with tile.TileContext(nc) as tc:
        with (
            tc.tile_pool(name="const", bufs=1) as const_p,
            tc.tile_pool(name="xt", bufs=1) as xt_p,
            tc.tile_pool(name="w1v", bufs=2) as w1v_p,
            tc.tile_pool(name="wqk", bufs=8) as wqk_p,
            tc.tile_pool(name="qkt", bufs=4) as qkt_p,
            tc.tile_pool(name="vsb", bufs=1) as vsb_p,
            tc.tile_pool(name="pt", bufs=6) as pt_p,
            tc.tile_pool(name="rc", bufs=2) as rc_p,
            tc.tile_pool(name="at", bufs=1) as at_p,
            tc.tile_pool(name="w2sb", bufs=1) as w2sb_p,
            tc.tile_pool(name="outsb", bufs=3) as out_p,
            tc.tile_pool(name="mmps", bufs=2, space="PSUM") as mmps_p,
            tc.tile_pool(name="scps", bufs=2, space="PSUM") as scps_p,
            tc.tile_pool(name="avps", bufs=2, space="PSUM") as avps_p,
        ):
            # ---------- prefetch DMAs; per-queue FIFO order == need order ----
            xt_sb = xt_p.tile([128, ND, S], BF16, tag="xt")
            wqks = [wqk_p.tile([128, ND, 2, 128], BF16, tag="wqk",
                               name=f"wqk{i}") for i in range(ND)]
            w1v0 = w1v_p.tile([128, ND, 512], BF16, tag="w1v", name="w1v0")
            w1v1 = w1v_p.tile([128, ND, 512], BF16, tag="w1v", name="w1v1")
            w2sb = w2sb_p.tile([128, ND, D], BF16, tag="w2sb")
            va_sb = vsb_p.tile([128, NT_S, H, 128], BF16, tag="va")
            at_sb = at_p.tile([128, ND, S], BF16, tag="at")

            # 3 DMA queues: sync(SP), scalar(ACT), gpsimd. FIFO per queue,
            # ordered by first use: QK0 needs xt+wqk0, then V needs w1v0.
            nc.sync.dma_start(xt_sb[:, 0:4, :], xt_lo[...])
            nc.scalar.dma_start(wqks[0][:], wqk_pk[0])
            nc.gpsimd.dma_start(xt_sb[:, 4:8, :], xt_hi[...])
            nc.sync.dma_start(w1v0[:, 0:4, :], w1v0_d[:, 0:4, :])
            nc.scalar.dma_start(w1v0[:, 4:8, :], w1v0_d[:, 4:8, :])

            # small consts early on the gpsimd queue (bv needed by first V add)
            bqk = const_p.tile([128, 2 * ND], F32)
            nc.gpsimd.dma_start(bqk[:], _dram_ap(b1, 0, [[1, 128], [128, 2 * ND]]))
            msk_sb = const_p.tile([128, NT_S], F32)
            nc.gpsimd.dma_start(msk_sb[:], _dram_ap(msk, 0, [[1, 128], [128, NT_S]]))
            bv = const_p.tile([128, D], F32)
            nc.gpsimd.dma_start(out=bv[:], in_=b1[None, 2 * D : 3 * D].to_broadcast([128, D]))

            for i in (2, 4, 6):
                nc.sync.dma_start(wqks[i][:], wqk_pk[i])
            nc.gpsimd.dma_start(wqks[1][:], wqk_pk[1])
            bo = const_p.tile([128, D], F32)
            nc.gpsimd.dma_start(out=bo[:], in_=b2[None, :].to_broadcast([128, D]))
            for i in (3, 5, 7):
                nc.gpsimd.dma_start(wqks[i][:], wqk_pk[i])
            nc.scalar.dma_start(w1v1[:], w1v1_d[...])
            nc.scalar.dma_start(w2sb[:], w2_d[...])

            # ones fill for the denominator half-blocks of va (V writes
            # later overwrite the V halves)
            nc.gpsimd.memset(va_sb[:], 1.0)
            # keep[k, q] = 1 if q >= k else 0, replicated for both heads
            tril2 = const_p.tile([128, 2, 128], BF16)
            make_upper_triangular(nc, tril2[:, 0, :], val=1.0, diag=True)
            make_upper_triangular(nc, tril2[:, 1, :], val=1.0, diag=True)
            pad_bias = const_p.tile([128, NT_S], F32)
            nc.vector.tensor_scalar(
                out=pad_bias[:], in0=msk_sb[:], scalar1=1.0, scalar2=1e9,
                op0=mybir.AluOpType.subtract, op1=mybir.AluOpType.mult,
            )

            # ---------------- emit helpers ----------------
            def emit_qk_pass(i):
                psq = mmps_p.tile([128, 512], F32, tag="mmps")
                psk = mmps_p.tile([128, 512], F32, tag="mmps")
                for d in range(ND):
                    nc.tensor.matmul(psq[:], wqks[i][:, d, 0, :], xt_sb[:, d, :],
                                     start=(d == 0), stop=(d == ND - 1))
                    nc.tensor.matmul(psk[:], wqks[i][:, d, 1, :], xt_sb[:, d, :],
                                     start=(d == 0), stop=(d == ND - 1))
                qt = qkt_p.tile([128, S], BF16, tag="qkt")
                kt = qkt_p.tile([128, S], BF16, tag="qkt")
                nc.scalar.activation(qt[:], psq[:], Ident,
                                     bias=bqk[:, i : i + 1], scale=1.0)
                nc.scalar.activation(kt[:], psk[:], Ident,
                                     bias=bqk[:, ND + i : ND + i + 1], scale=1.0)
                return qt, kt

            def emit_v(c, w1v_c):
                # V projection c-half: heads 8c..8c+7 into [V|1]/[1|V] blocks
                for t in range(NT_S):
                    ps = mmps_p.tile([128, 512], F32, tag="mmps")
                    for d in range(ND):
                        nc.tensor.matmul(ps[:], xt_sb[:, d, t * 128 : (t + 1) * 128],
                                         w1v_c[:, d, :],
                                         start=(d == 0), stop=(d == ND - 1))
                    # one strided add: psum cols (pair, half, 64) + bias ->
                    # va cols 256p + 192*half + j (even head low, odd head high)
                    vv = va_sb[:, t, :, :]
                    dst = bass.AP(tensor=vv.tensor, offset=vv.offset + c * 1024,
                                  ap=[vv.ap[0], [256, 4], [192, 2], [1, 64]])
                    pp = ps[:]
                    src = bass.AP(tensor=pp.tensor, offset=pp.offset,
                                  ap=[pp.ap[0], [128, 4], [64, 2], [1, 64]])
                    bb = bv[:]
                    bsrc = bass.AP(tensor=bb.tensor, offset=bb.offset + c * 512,
                                   ap=[bb.ap[0], [128, 4], [64, 2], [1, 64]])
                    # (gpsimd cannot read PSUM -> all psum-draining ops on DVE)
                    nc.vector.tensor_tensor(out=dst, in0=src, in1=bsrc, op=ADD)

            def emit_scores(i, qt, kt):
                pts = []
                for sk in range(NT_S):
                    w = S - sk * 128
                    sc = scps_p.tile([128, 2, 512], F32, tag="scps")
                    nc.tensor.matmul(sc[:, 0, 0:w], kt[0:64, sk * 128 : (sk + 1) * 128],
                                     qt[0:64, sk * 128 : S], start=True, stop=True)
                    nc.tensor.matmul(sc[:, 1, 0:w], kt[64:128, sk * 128 : (sk + 1) * 128],
                                     qt[64:128, sk * 128 : S], start=True, stop=True)
                    pt = pt_p.tile([128, 2, 512], BF16, tag="pt")
                    nc.scalar.activation(pt[:, :, 0:w], sc[:, :, 0:w], Exp,
                                         bias=pad_bias[:, sk : sk + 1], scale=SCALE)
                    # zero strictly-future entries of the diagonal block
                    nc.vector.tensor_tensor(out=pt[:, :, 0:128], in0=pt[:, :, 0:128],
                                            in1=tril2[:], op=MUL)
                    pts.append(pt)
                return pts

            def emit_av(i, pts):
                h_e, h_o = 2 * i, 2 * i + 1
                av_e = avps_p.tile([128, 512], F32, tag="avps")
                av_o = avps_p.tile([128, 512], F32, tag="avps")
                for sk in range(NT_S):
                    w = S - sk * 128
                    nc.tensor.matmul(av_e[:, sk * 128 : S], va_sb[:, sk, h_e, :],
                                     pts[sk][:, 0, 0:w],
                                     start=(sk == 0), stop=(sk == NT_S - 1))
                    nc.tensor.matmul(av_o[:, sk * 128 : S], va_sb[:, sk, h_o, :],
                                     pts[sk][:, 1, 0:w],
                                     start=(sk == 0), stop=(sk == NT_S - 1))
                # normalize: attn rows * recip(denominator rows)
                rc = rc_p.tile([128, 512], F32, tag="rc")
                nc.vector.reciprocal_approx_fast(out=rc[0:64, :], in_=av_e[64:128, :])
                nc.vector.tensor_tensor(out=at_sb[0:64, i, :], in0=av_e[0:64, :],
                                        in1=rc[0:64, :], op=MUL)
                nc.vector.reciprocal_approx_fast(out=rc[64:128, :], in_=av_o[0:64, :])
                nc.vector.tensor_tensor(out=at_sb[64:128, i, :], in0=av_o[64:128, :],
                                        in1=rc[64:128, :], op=MUL)

            # ---------------- schedule ----------------
            qts = {0: emit_qk_pass(0)}
            emit_v(0, w1v0)
            for i in range(ND):
                qt, kt = qts.pop(i)
                pts = emit_scores(i, qt, kt)
                # independent PE work between scores and A^T hides the exps
                if i == 0:
                    qts[1] = emit_qk_pass(1)
                elif i == 1:
                    emit_v(1, w1v1)
                elif i + 1 < ND:
                    qts[i + 1] = emit_qk_pass(i + 1)
                emit_av(i, pts)
                if i == 1:
                    qts[2] = emit_qk_pass(2)

            # ---------------- c_proj ----------------
            for t in range(NT_S):
                for c in range(2):
                    ps = mmps_p.tile([128, 512], F32, tag="mmps")
                    for d in range(ND):
                        nc.tensor.matmul(ps[:], at_sb[:, d, t * 128 : (t + 1) * 128],
                                         w2sb[:, d, c * 512 : (c + 1) * 512],
                                         start=(d == 0), stop=(d == ND - 1))
                    ob = out_p.tile([128, 512], F32, tag="outsb")
                    nc.vector.tensor_tensor(out=ob[:], in0=ps[:],
                                            in1=bo[:, c * 512 : (c + 1) * 512],
                                            op=ADD)
                    nc.scalar.dma_start(
                        out[t * 128 : (t + 1) * 128, c * 512 : (c + 1) * 512], ob[:]
                    )

    nc.compile()
    return nc


def _get_nc():
    if "nc" not in _CACHED:
        _CACHED["nc"] = _build_nc()
    return _CACHED["nc"]


def _pack_weights(w1, w2):
    """Host-side bf16 packing into [128, ...] contiguous DMA blocks."""
    w1r = w1.reshape(ND, 128, 3 * D)
    qs = w1r[:, :, :D].reshape(ND, 128, ND, 128)       # [d, p, i, c]
    ks = w1r[:, :, D : 2 * D].reshape(ND, 128, ND, 128)
    pk = np.stack([qs, ks], axis=3)                    # [d, p, i, {q,k}, c]
    wqk_pk = pk.transpose(2, 1, 0, 3, 4).astype(NPBF16)  # [i, p, d, {q,k}, c]
    w1v = w1r[:, :, 2 * D :]                           # [d, p, 1024]
    w1v0 = w1v[:, :, :512].transpose(1, 0, 2).astype(NPBF16)   # [p, d, 512]
    w1v1 = w1v[:, :, 512:].transpose(1, 0, 2).astype(NPBF16)
    w2pk = w2.reshape(ND, 128, D).transpose(1, 0, 2).astype(NPBF16)  # [p, d, e]
    return (np.ascontiguousarray(wqk_pk), np.ascontiguousarray(w1v0),
            np.ascontiguousarray(w1v1), np.ascontiguousarray(w2pk))


def _make_in_maps(x, mask, w1, b1, w2, b2):
    wqk_pk, w1v0, w1v1, w2pk = _pack_weights(w1, w2)
    in_maps = []
    for b in range(N_CORES):
        xtp = x[b].T.reshape(ND, 128, S).transpose(1, 0, 2)  # [p, d, s]
        in_maps.append({
            "xt_lo": np.ascontiguousarray(xtp[:, 0:4].astype(NPBF16)),
            "xt_hi": np.ascontiguousarray(xtp[:, 4:8].astype(NPBF16)),
            "w1v0": w1v0, "w1v1": w1v1, "wqk_pk": wqk_pk, "w2pk": w2pk,
            "msk": np.ascontiguousarray(mask[b]),
            "b1": b1, "b2": b2,
        })
    return in_maps


def _trace_setup(inputs):
    """Build (nc, in_maps) exactly as kernel() would — for test.py tracing."""
    x = np.asarray(inputs["x"], dtype=np.float32)
    mask = np.asarray(inputs["mask"], dtype=np.float32)
    w1 = np.ascontiguousarray(np.asarray(inputs["c_attn_w"], dtype=np.float32))
    b1 = np.ascontiguousarray(np.asarray(inputs["c_attn_b"], dtype=np.float32))
    w2 = np.ascontiguousarray(np.asarray(inputs["c_proj_w"], dtype=np.float32))
    b2 = np.ascontiguousarray(np.asarray(inputs["c_proj_b"], dtype=np.float32))
    return _get_nc(), _make_in_maps(x, mask, w1, b1, w2, b2)


def kernel(x, mask, c_attn_w, c_attn_b, c_proj_w, c_proj_b):
    x = np.asarray(x, dtype=np.float32)
    mask = np.asarray(mask, dtype=np.float32)
    w1 = np.ascontiguousarray(np.asarray(c_attn_w, dtype=np.float32))
    b1 = np.ascontiguousarray(np.asarray(c_attn_b, dtype=np.float32))
    w2 = np.ascontiguousarray(np.asarray(c_proj_w, dtype=np.float32))
    b2 = np.ascontiguousarray(np.asarray(c_proj_b, dtype=np.float32))

    nc = _get_nc()
    in_maps = _make_in_maps(x, mask, w1, b1, w2, b2)
    res = run_bass_kernel_spmd(nc, in_maps, list(range(N_CORES)))
    return np.stack([res.results[b]["out"] for b in range(N_CORES)], axis=0)


# revision 39
# speedup vs baseline: 1.5050x; 1.1072x over previous
"""Trainium2 Bass kernel for fused multi-head causal attention (GPT-2 style).

Full-input contract: kernel(**inputs) takes the complete tensors and returns
the complete output. Internally: data-parallel over the batch dim (B=8) across
8 NeuronCores; each core runs the whole attention block for one batch element.

Per-core dataflow (S=512, D=1024, H=16, dh=64). All matmul operands are bf16
(PSUM accumulation stays fp32); rel-err tolerance is 2e-2 so bf16 rounding
(~0.5%) is safe and doubles effective PE/DMA throughput vs fp32:

  x^T [128, d, S]   host-packed bf16, resident in SBUF (2 DMA queues)
  V:   psum[s,n]   = x^T[:,s].T @ W1v        -> [V|1]/[1|V] head blocks in SBUF
  QK:  psum[n,s]   = W1qk[:,n].T @ x^T       -> Q^T,K^T   (bias on ACT engine)
  S^T: psum[sk,sq] = K_h^T[:,sk].T @ Q_h^T   (scores transposed, causal-trimmed)
  P^T  = exp(S^T/8 + pad_bias) on ACT; tril multiply on DVE zeroes the future
  A^T: psum = [V_h|1].T @ P^T                (64 attn rows + 64 denominator rows)
  A^T_norm = psum_attn * recip_approx(psum_denom)  -> A^T bf16 tiles [n, s]
  out: psum[s,e]   = A^T[:,s].T @ W2         + b2 (fp32 out)

Schedule is ordered by DMA arrival: QK pass 0 (needs x^T + 0.5MB of weights)
runs first, then the V projection c-half 0; head pairs are software-pipelined
with the next QK pass (or V c-half 1) emitted between a pair's score matmuls
and its A^T matmuls so the PE never waits on the ACT-engine exps.
"""

import sys

if "/opt/trn_rl_repo" not in sys.path:
    sys.path.insert(0, "/opt/trn_rl_repo")

import numpy as np

import concourse.bass as bass
import concourse.mybir as mybir
import concourse.tile as tile
from concourse import bacc
from concourse.bass_utils import run_bass_kernel_spmd
from concourse.masks import make_upper_triangular

F32 = mybir.dt.float32
BF16 = mybir.dt.bfloat16
NPBF16 = mybir.dt.np(BF16)

B, S, D = 8, 512, 1024
H = 16
DH = D // H          # 64
NT_S = S // 128      # 4 s-tiles
ND = D // 128        # 8 d-tiles
N_CORES = 8
SCALE = 1.0 / 8.0    # 1/sqrt(head_dim)

_CACHED = {}


def _dram_ap(t, offset, dims):
    """Raw strided DRAM access pattern ([step, count] pairs, elements)."""
    return bass.AP(tensor=t[...].tensor, offset=offset, ap=dims)


def _build_nc(dbg=False):
    nc = bacc.Bacc("TRN2", target_bir_lowering=False, debug=False)

    Exp = mybir.ActivationFunctionType.Exp
    Ident = mybir.ActivationFunctionType.Identity
    ADD = mybir.AluOpType.add
    MUL = mybir.AluOpType.mult

    dbg_d = {}
    if dbg:
        dbg_d["dva"] = nc.dram_tensor("dva", [128, NT_S, D], BF16,
                                      kind="ExternalOutput")
        dbg_d["dat"] = nc.dram_tensor("dat", [128, ND, S], BF16,
                                      kind="ExternalOutput")
        dbg_d["dqt"] = nc.dram_tensor("dqt", [128, S], BF16, kind="ExternalOutput")
        dbg_d["dkt"] = nc.dram_tensor("dkt", [128, S], BF16, kind="ExternalOutput")
        dbg_d["dpt"] = nc.dram_tensor("dpt", [NT_S, 128, 2, 512], BF16,
                                      kind="ExternalOutput")

    # host-packed bf16 inputs (all [128 partitions, ...] contiguous)
    xt_lo = nc.dram_tensor("xt_lo", [128, 4, S], BF16, kind="ExternalInput")
    xt_hi = nc.dram_tensor("xt_hi", [128, 4, S], BF16, kind="ExternalInput")
    w1v0_d = nc.dram_tensor("w1v0", [128, ND, 512], BF16, kind="ExternalInput")
    w1v1_d = nc.dram_tensor("w1v1", [128, ND, 512], BF16, kind="ExternalInput")
    wqk_pk = nc.dram_tensor("wqk_pk", [ND, 128, ND, 2, 128], BF16,
                            kind="ExternalInput")
    w2_d = nc.dram_tensor("w2pk", [128, ND, D], BF16, kind="ExternalInput")
    # host-packed [128, n] consts — a strided gather here costs 128 DMA
    # descriptors and clogs the DMA engines at startup
    msk_d = nc.dram_tensor("mskpk", [128, NT_S], F32, kind="ExternalInput")
    bqk_d = nc.dram_tensor("bqkpk", [128, 2 * ND], F32, kind="ExternalInput")
    bv_d = nc.dram_tensor("bvpk", [128, D], BF16, kind="ExternalInput")
    bo_d = nc.dram_tensor("bopk", [128, D], F32, kind="ExternalInput")
    out = nc.dram_tensor("out", [S, D], F32, kind="ExternalOutput")

    with tile.TileContext(nc) as tc:
        with (
            tc.tile_pool(name="const", bufs=1) as const_p,
            tc.tile_pool(name="xt", bufs=1) as xt_p,
            tc.tile_pool(name="w1v", bufs=2) as w1v_p,
            tc.tile_pool(name="wqk", bufs=8) as wqk_p,
            tc.tile_pool(name="qkt", bufs=6) as qkt_p,
            tc.tile_pool(name="vsb", bufs=1) as vsb_p,
            tc.tile_pool(name="pt", bufs=9) as pt_p,
            tc.tile_pool(name="rc", bufs=2) as rc_p,
            tc.tile_pool(name="at", bufs=1) as at_p,
            tc.tile_pool(name="w2sb", bufs=1) as w2sb_p,
            tc.tile_pool(name="outsb", bufs=3) as out_p,
            tc.tile_pool(name="mmps", bufs=2, space="PSUM") as mmps_p,
            tc.tile_pool(name="scps", bufs=2, space="PSUM") as scps_p,
            tc.tile_pool(name="avps", bufs=1, space="PSUM") as avps_p,
            tc.tile_pool(name="dnps", bufs=1, space="PSUM") as dnps_p,
        ):
            # ---------- prefetch DMAs; per-queue FIFO order == need order ----
            xt_sb = xt_p.tile([128, ND, S], BF16, tag="xt")
            wqks = [wqk_p.tile([128, ND, 2, 128], BF16, tag="wqk",
                               name=f"wqk{i}") for i in range(ND)]
            w1v0 = w1v_p.tile([128, ND, 512], BF16, tag="w1v", name="w1v0")
            w1v1 = w1v_p.tile([128, ND, 512], BF16, tag="w1v", name="w1v1")
            w2sb = w2sb_p.tile([128, ND, D], BF16, tag="w2sb")
            va_sb = vsb_p.tile([128, NT_S, D], BF16, tag="va")
            at_sb = at_p.tile([128, ND, S], BF16, tag="at")

            # ACT-table warm-up: first Exp otherwise pays ~1.3us table load
            warm = const_p.tile([128, 1], F32)
            nc.vector.memset(warm[:], 0.0)
            nc.scalar.activation(warm[:], warm[:],
                                 mybir.ActivationFunctionType.Exp)

            # 3 DMA queues: sync(SP), scalar(ACT), gpsimd. FIFO per queue,
            # ordered by first use: QK0/QK1 need xt+wqk, then V needs w1v0.
            bqk = const_p.tile([128, 2 * ND], F32)
            msk_sb = const_p.tile([128, NT_S], F32)
            bv = const_p.tile([128, D], BF16)
            bo = const_p.tile([128, D], F32)
            nc.gpsimd.dma_start(bqk[:], bqk_d[...])
            nc.gpsimd.dma_start(msk_sb[:], msk_d[...])
            nc.sync.dma_start(xt_sb[:, 0:4, :], xt_lo[...])
            nc.scalar.dma_start(wqks[0][:], wqk_pk[0])
            nc.gpsimd.dma_start(xt_sb[:, 4:8, :], xt_hi[...])
            nc.sync.dma_start(wqks[1][:], wqk_pk[1])
            nc.sync.dma_start(w1v0[:, 0:4, :], w1v0_d[:, 0:4, :])
            nc.scalar.dma_start(w1v0[:, 4:8, :], w1v0_d[:, 4:8, :])
            nc.gpsimd.dma_start(bv[:], bv_d[...])

            for i in (2, 4, 6):
                nc.sync.dma_start(wqks[i][:], wqk_pk[i])
            for i in (3, 5, 7):
                nc.gpsimd.dma_start(wqks[i][:], wqk_pk[i])
            nc.scalar.dma_start(w1v1[:], w1v1_d[...])
            nc.scalar.dma_start(w2sb[:], w2_d[...])
            nc.scalar.dma_start(bo[:], bo_d[...])

            # ones column block for the denominator matmuls
            ones64 = const_p.tile([128, 64], BF16)
            nc.gpsimd.memset(ones64[:], 1.0)
            # keep[k, q] = 1 if q >= k else 0, replicated for both heads
            tril2 = const_p.tile([128, 2, 128], BF16)
            make_upper_triangular(nc, tril2[:, 0, :], val=1.0, diag=True)
            make_upper_triangular(nc, tril2[:, 1, :], val=1.0, diag=True)
            pad_bias = const_p.tile([128, NT_S], F32)
            nc.vector.tensor_scalar(
                out=pad_bias[:], in0=msk_sb[:], scalar1=1.0, scalar2=1e9,
                op0=mybir.AluOpType.subtract, op1=mybir.AluOpType.mult,
            )

            # ---------------- emit helpers ----------------
            def emit_qk_pass(i):
                psq = mmps_p.tile([128, 512], F32, tag="mmps")
                psk = mmps_p.tile([128, 512], F32, tag="mmps")
                for d in range(ND):
                    nc.tensor.matmul(psq[:], wqks[i][:, d, 0, :], xt_sb[:, d, :],
                                     start=(d == 0), stop=(d == ND - 1))
                    nc.tensor.matmul(psk[:], wqks[i][:, d, 1, :], xt_sb[:, d, :],
                                     start=(d == 0), stop=(d == ND - 1))
                qt = qkt_p.tile([128, S], BF16, tag="qkt")
                kt = qkt_p.tile([128, S], BF16, tag="qkt")
                nc.scalar.activation(qt[:], psq[:], Ident,
                                     bias=bqk[:, i : i + 1], scale=1.0)
                nc.scalar.activation(kt[:], psk[:], Ident,
                                     bias=bqk[:, ND + i : ND + i + 1], scale=1.0)
                return qt, kt

            def emit_v(c, w1v_c):
                # V projection c-half: heads 8c..8c+7 -> va cols c*512..+512
                for t in range(NT_S):
                    ps = mmps_p.tile([128, 512], F32, tag="mmps")
                    for d in range(ND):
                        nc.tensor.matmul(ps[:], xt_sb[:, d, t * 128 : (t + 1) * 128],
                                         w1v_c[:, d, :],
                                         start=(d == 0), stop=(d == ND - 1))
                    # (gpsimd cannot read PSUM -> all psum-draining ops on DVE)
                    nc.vector.tensor_tensor(
                        out=va_sb[:, t, c * 512 : (c + 1) * 512], in0=ps[:],
                        in1=bv[:, c * 512 : (c + 1) * 512], op=ADD)

            def emit_scores(i, qt, kt):
                pts = []
                for sk in range(NT_S):
                    w = S - sk * 128
                    sc = scps_p.tile([128, 2, 512], F32, tag="scps")
                    nc.tensor.matmul(sc[:, 0, 0:w], kt[0:64, sk * 128 : (sk + 1) * 128],
                                     qt[0:64, sk * 128 : S], start=True, stop=True)
                    nc.tensor.matmul(sc[:, 1, 0:w], kt[64:128, sk * 128 : (sk + 1) * 128],
                                     qt[64:128, sk * 128 : S], start=True, stop=True)
                    pt = pt_p.tile([128, 2, 512], BF16, tag="pt")
                    nc.scalar.activation(pt[:, :, 0:w], sc[:, :, 0:w], Exp,
                                         bias=pad_bias[:, sk : sk + 1], scale=SCALE)
                    # zero strictly-future entries of the diagonal block
                    nc.vector.tensor_tensor(out=pt[:, :, 0:128], in0=pt[:, :, 0:128],
                                            in1=tril2[:], op=MUL)
                    pts.append(pt)
                return pts

            def emit_av(i, pts):
                h_e, h_o = 2 * i, 2 * i + 1
                # packed denominators: den_e on partitions 0:64, den_o on
                # 64:128 of ONE tile -> a single unshifted reciprocal
                dn = dnps_p.tile([128, 512], F32, tag="dnps")
                av2 = avps_p.tile([128, 512], F32, tag="avps")
                for sk in range(NT_S):
                    w = S - sk * 128
                    nc.tensor.matmul(dn[0:64, sk * 128 : S], ones64[:],
                                     pts[sk][:, 0, 0:w],
                                     start=(sk == 0), stop=(sk == NT_S - 1))
                    nc.tensor.matmul(dn[64:128, sk * 128 : S], ones64[:],
                                     pts[sk][:, 1, 0:w],
                                     start=(sk == 0), stop=(sk == NT_S - 1))
                # chunked recip+normalize for the last pairs: c_proj reads
                # at[:, i, :] in 128-col slices, so let them unblock
                # progressively (these norms sit in the c_proj critical path)
                nch = 4 if i >= ND - 2 else 1
                cw = 512 // nch
                rc = rc_p.tile([128, 512], F32, tag="rc")
                for ch in range(nch):
                    nc.vector.reciprocal(out=rc[:, ch * cw : (ch + 1) * cw],
                                         in_=dn[:, ch * cw : (ch + 1) * cw])
                for sk in range(NT_S):
                    w = S - sk * 128
                    nc.tensor.matmul(av2[0:64, sk * 128 : S],
                                     va_sb[:, sk, h_e * 64 : h_e * 64 + 64],
                                     pts[sk][:, 0, 0:w],
                                     start=(sk == 0), stop=(sk == NT_S - 1))
                    nc.tensor.matmul(av2[64:128, sk * 128 : S],
                                     va_sb[:, sk, h_o * 64 : h_o * 64 + 64],
                                     pts[sk][:, 1, 0:w],
                                     start=(sk == 0), stop=(sk == NT_S - 1))
                for ch in range(nch):
                    sl = slice(ch * cw, (ch + 1) * cw)
                    nc.vector.tensor_tensor(out=at_sb[:, i, sl],
                                            in0=av2[:, sl], in1=rc[:, sl],
                                            op=MUL)

            # ---------------- schedule ----------------
            # Front-load QK passes 0-2 + scores(0) so the PE has work while
            # the V weights stream in; V_c0 right before the pair loop.
            qt0, kt0 = emit_qk_pass(0)
            qts = {1: emit_qk_pass(1)}
            if dbg:
                nc.sync.dma_start(dbg_d["dqt"][...], qt0[:])
                nc.sync.dma_start(dbg_d["dkt"][...], kt0[:])
            pts0 = emit_scores(0, qt0, kt0)
            if dbg:
                for sk in range(NT_S):
                    nc.sync.dma_start(dbg_d["dpt"][sk], pts0[sk][:])
            qts[2] = emit_qk_pass(2)
            emit_v(0, w1v0)
            next_qk = 3
            for i in range(ND):
                pts = pts0 if i == 0 else emit_scores(i, *qts.pop(i))
                # independent PE work between scores and A^T hides the exps;
                # V_c1 at i=3: late enough for its DMA, before pair 4 needs it
                if i == 3:
                    emit_v(1, w1v1)
                elif i >= 1 and next_qk < ND:
                    qts[next_qk] = emit_qk_pass(next_qk)
                    next_qk += 1
                elif i == ND - 1:
                    # hide the last pair's exps + reciprocal: run the first
                    # two c_proj chains' d=0..6 steps now, d=7 after norm(7)
                    cpre = []
                    for c in range(2):
                        ps = mmps_p.tile([128, 512], F32, tag="mmps")
                        for d in range(ND - 1):
                            nc.tensor.matmul(ps[:], at_sb[:, d, 0:128],
                                             w2sb[:, d, c * 512 : (c + 1) * 512],
                                             start=(d == 0), stop=False)
                        cpre.append(ps)
                emit_av(i, pts)

            if dbg:
                nc.sync.dma_start(dbg_d["dva"][...], va_sb[:])
                nc.sync.dma_start(dbg_d["dat"][...], at_sb[:])

            # ---------------- c_proj ----------------
            for t in range(NT_S):
                for c in range(2):
                    if t == 0:
                        ps = cpre[c]
                        nc.tensor.matmul(ps[:], at_sb[:, ND - 1, 0:128],
                                         w2sb[:, ND - 1, c * 512 : (c + 1) * 512],
                                         start=False, stop=True)
                    else:
                        ps = mmps_p.tile([128, 512], F32, tag="mmps")
                        for d in range(ND):
                            nc.tensor.matmul(ps[:],
                                             at_sb[:, d, t * 128 : (t + 1) * 128],
                                             w2sb[:, d, c * 512 : (c + 1) * 512],
                                             start=(d == 0), stop=(d == ND - 1))
                    ob = out_p.tile([128, 512], F32, tag="outsb")
                    nc.vector.tensor_tensor(out=ob[:], in0=ps[:],
                                            in1=bo[:, c * 512 : (c + 1) * 512],
                                            op=ADD)
                    # sync engine is idle at the tail; ACT would delay these
                    # issues behind the queued exps
                    nc.sync.dma_start(
                        out[t * 128 : (t + 1) * 128, c * 512 : (c + 1) * 512], ob[:]
                    )

    nc.compile()
    return nc


def _get_nc(dbg=False):
    key = ("nc", dbg)
    if key not in _CACHED:
        _CACHED[key] = _build_nc(dbg)
    return _CACHED[key]


def _pack_weights(w1, w2):
    """Host-side bf16 packing into [128, ...] contiguous DMA blocks."""
    w1r = w1.reshape(ND, 128, 3 * D)
    qs = w1r[:, :, :D].reshape(ND, 128, ND, 128)       # [d, p, i, c]
    ks = w1r[:, :, D : 2 * D].reshape(ND, 128, ND, 128)
    pk = np.stack([qs, ks], axis=3)                    # [d, p, i, {q,k}, c]
    wqk_pk = pk.transpose(2, 1, 0, 3, 4).astype(NPBF16)  # [i, p, d, {q,k}, c]
    w1v = w1r[:, :, 2 * D :]                           # [d, p, 1024]
    w1v0 = w1v[:, :, :512].transpose(1, 0, 2).astype(NPBF16)   # [p, d, 512]
    w1v1 = w1v[:, :, 512:].transpose(1, 0, 2).astype(NPBF16)
    w2pk = w2.reshape(ND, 128, D).transpose(1, 0, 2).astype(NPBF16)  # [p, d, e]
    return (np.ascontiguousarray(wqk_pk), np.ascontiguousarray(w1v0),
            np.ascontiguousarray(w1v1), np.ascontiguousarray(w2pk))


def _make_in_maps(x, mask, w1, b1, w2, b2):
    wqk_pk, w1v0, w1v1, w2pk = _pack_weights(w1, w2)
    bqk_pk = np.ascontiguousarray(b1[: 2 * D].reshape(2 * ND, 128).T)
    bv_pk = np.ascontiguousarray(
        np.broadcast_to(b1[2 * D :], (128, D)).astype(NPBF16))
    bo_pk = np.ascontiguousarray(np.broadcast_to(b2, (128, D)))
    in_maps = []
    for b in range(N_CORES):
        xtp = x[b].T.reshape(ND, 128, S).transpose(1, 0, 2)  # [p, d, s]
        in_maps.append({
            "xt_lo": np.ascontiguousarray(xtp[:, 0:4].astype(NPBF16)),
            "xt_hi": np.ascontiguousarray(xtp[:, 4:8].astype(NPBF16)),
            "w1v0": w1v0, "w1v1": w1v1, "wqk_pk": wqk_pk, "w2pk": w2pk,
            "mskpk": np.ascontiguousarray(mask[b].reshape(NT_S, 128).T),
            "bqkpk": bqk_pk, "bvpk": bv_pk, "bopk": bo_pk,
        })
    return in_maps


def _trace_setup(inputs):
    """Build (nc, in_maps) exactly as kernel() would — for test.py tracing."""
    x = np.asarray(inputs["x"], dtype=np.float32)
    mask = np.asarray(inputs["mask"], dtype=np.float32)
    w1 = np.ascontiguousarray(np.asarray(inputs["c_attn_w"], dtype=np.float32))
    b1 = np.ascontiguousarray(np.asarray(inputs["c_attn_b"], dtype=np.float32))
    w2 = np.ascontiguousarray(np.asarray(inputs["c_proj_w"], dtype=np.float32))
    b2 = np.ascontiguousarray(np.asarray(inputs["c_proj_b"], dtype=np.float32))
    return _get_nc(), _make_in_maps(x, mask, w1, b1, w2, b2)


def kernel(x, mask, c_attn_w, c_attn_b, c_proj_w, c_proj_b):
    x = np.asarray(x, dtype=np.float32)
    mask = np.asarray(mask, dtype=np.float32)
    w1 = np.ascontiguousarray(np.asarray(c_attn_w, dtype=np.float32))
    b1 = np.ascontiguousarray(np.asarray(c_attn_b, dtype=np.float32))
    w2 = np.ascontiguousarray(np.asarray(c_proj_w, dtype=np.float32))
    b2 = np.ascontiguousarray(np.asarray(c_proj_b, dtype=np.float32))

    nc = _get_nc()
    in_maps = _make_in_maps(x, mask, w1, b1, w2, b2)
    res = run_bass_kernel_spmd(nc, in_maps, list(range(N_CORES)))
    return np.stack([res.results[b]["out"] for b in range(N_CORES)], axis=0)


# revision 41
# speedup vs baseline: 1.5439x; 1.0258x over previous
"""Trainium2 Bass kernel for fused multi-head causal attention (GPT-2 style).

Full-input contract: kernel(**inputs) takes the complete tensors and returns
the complete output. Internally: data-parallel over the batch dim (B=8) across
8 NeuronCores; each core runs the whole attention block for one batch element.

Per-core dataflow (S=512, D=1024, H=16, dh=64). All matmul operands are bf16
(PSUM accumulation stays fp32); rel-err tolerance is 2e-2 so bf16 rounding
(~0.5%) is safe and doubles effective PE/DMA throughput vs fp32:

  x^T [128, d, S]   host-packed bf16, resident in SBUF (2 DMA queues)
  V:   psum[s,n]   = x^T[:,s].T @ W1v        -> [V|1]/[1|V] head blocks in SBUF
  QK:  psum[n,s]   = W1qk[:,n].T @ x^T       -> Q^T,K^T   (bias on ACT engine)
  S^T: psum[sk,sq] = K_h^T[:,sk].T @ Q_h^T   (scores transposed, causal-trimmed)
  P^T  = exp(S^T/8 + pad_bias) on ACT; tril multiply on DVE zeroes the future
  A^T: psum = [V_h|1].T @ P^T                (64 attn rows + 64 denominator rows)
  A^T_norm = psum_attn * recip_approx(psum_denom)  -> A^T bf16 tiles [n, s]
  out: psum[s,e]   = A^T[:,s].T @ W2         + b2 (fp32 out)

Schedule is ordered by DMA arrival: QK pass 0 (needs x^T + 0.5MB of weights)
runs first, then the V projection c-half 0; head pairs are software-pipelined
with the next QK pass (or V c-half 1) emitted between a pair's score matmuls
and its A^T matmuls so the PE never waits on the ACT-engine exps.
"""

import sys

if "/opt/trn_rl_repo" not in sys.path:
    sys.path.insert(0, "/opt/trn_rl_repo")

import numpy as np

import concourse.bass as bass
import concourse.mybir as mybir
import concourse.tile as tile
from concourse import bacc
from concourse.bass_utils import run_bass_kernel_spmd
from concourse.masks import make_upper_triangular

F32 = mybir.dt.float32
BF16 = mybir.dt.bfloat16
NPBF16 = mybir.dt.np(BF16)

B, S, D = 8, 512, 1024
H = 16
DH = D // H          # 64
NT_S = S // 128      # 4 s-tiles
ND = D // 128        # 8 d-tiles
N_CORES = 8
SCALE = 1.0 / 8.0    # 1/sqrt(head_dim)

_CACHED = {}


def _dram_ap(t, offset, dims):
    """Raw strided DRAM access pattern ([step, count] pairs, elements)."""
    return bass.AP(tensor=t[...].tensor, offset=offset, ap=dims)


def _build_nc(dbg=False):
    nc = bacc.Bacc("TRN2", target_bir_lowering=False, debug=False)

    Exp = mybir.ActivationFunctionType.Exp
    Ident = mybir.ActivationFunctionType.Identity
    ADD = mybir.AluOpType.add
    MUL = mybir.AluOpType.mult

    dbg_d = {}
    if dbg:
        dbg_d["dva"] = nc.dram_tensor("dva", [128, NT_S, D], BF16,
                                      kind="ExternalOutput")
        dbg_d["dat"] = nc.dram_tensor("dat", [128, ND, S], BF16,
                                      kind="ExternalOutput")
        dbg_d["dqt"] = nc.dram_tensor("dqt", [128, S], BF16, kind="ExternalOutput")
        dbg_d["dkt"] = nc.dram_tensor("dkt", [128, S], BF16, kind="ExternalOutput")
        dbg_d["dpt"] = nc.dram_tensor("dpt", [NT_S, 128, 2, 512], BF16,
                                      kind="ExternalOutput")

    # host-packed bf16 inputs (all [128 partitions, ...] contiguous)
    xt_lo = nc.dram_tensor("xt_lo", [128, 4, S], BF16, kind="ExternalInput")
    xt_hi = nc.dram_tensor("xt_hi", [128, 4, S], BF16, kind="ExternalInput")
    w1v0_d = nc.dram_tensor("w1v0", [128, ND, 512], BF16, kind="ExternalInput")
    w1v1_d = nc.dram_tensor("w1v1", [128, ND, 512], BF16, kind="ExternalInput")
    wqk_pk = nc.dram_tensor("wqk_pk", [ND, 128, ND, 2, 128], BF16,
                            kind="ExternalInput")
    w2_d = nc.dram_tensor("w2pk", [128, ND, D], BF16, kind="ExternalInput")
    # host-packed [128, n] consts — a strided gather here costs 128 DMA
    # descriptors and clogs the DMA engines at startup
    msk_d = nc.dram_tensor("mskpk", [128, NT_S], F32, kind="ExternalInput")
    bqk_d = nc.dram_tensor("bqkpk", [128, 2 * ND], F32, kind="ExternalInput")
    bv_d = nc.dram_tensor("bvpk", [128, D], BF16, kind="ExternalInput")
    bo_d = nc.dram_tensor("bopk", [128, D], F32, kind="ExternalInput")
    out = nc.dram_tensor("out", [S, D], F32, kind="ExternalOutput")

    with tile.TileContext(nc) as tc:
        with (
            tc.tile_pool(name="const", bufs=1) as const_p,
            tc.tile_pool(name="xt", bufs=1) as xt_p,
            tc.tile_pool(name="w1v", bufs=2) as w1v_p,
            tc.tile_pool(name="wqk", bufs=8) as wqk_p,
            tc.tile_pool(name="qkt", bufs=6) as qkt_p,
            tc.tile_pool(name="vsb", bufs=1) as vsb_p,
            tc.tile_pool(name="pt", bufs=9) as pt_p,
            tc.tile_pool(name="rc", bufs=2) as rc_p,
            tc.tile_pool(name="at", bufs=1) as at_p,
            tc.tile_pool(name="w2sb", bufs=1) as w2sb_p,
            tc.tile_pool(name="outsb", bufs=3) as out_p,
            tc.tile_pool(name="mmps", bufs=2, space="PSUM") as mmps_p,
            tc.tile_pool(name="scps", bufs=2, space="PSUM") as scps_p,
            tc.tile_pool(name="avps", bufs=1, space="PSUM") as avps_p,
            tc.tile_pool(name="dnps", bufs=1, space="PSUM") as dnps_p,
        ):
            # ---------- prefetch DMAs; per-queue FIFO order == need order ----
            xt_sb = xt_p.tile([128, ND, S], BF16, tag="xt")
            wqks = [wqk_p.tile([128, ND, 2, 128], BF16, tag="wqk",
                               name=f"wqk{i}") for i in range(ND)]
            w1v0 = w1v_p.tile([128, ND, 512], BF16, tag="w1v", name="w1v0")
            w1v1 = w1v_p.tile([128, ND, 512], BF16, tag="w1v", name="w1v1")
            w2sb = w2sb_p.tile([128, ND, D], BF16, tag="w2sb")
            va_sb = vsb_p.tile([128, NT_S, D], BF16, tag="va")
            at_sb = at_p.tile([128, ND, S], BF16, tag="at")

            # ACT-table warm-up: first Exp otherwise pays ~1.3us table load
            warm = const_p.tile([128, 1], F32)
            nc.vector.memset(warm[:], 0.0)
            nc.scalar.activation(warm[:], warm[:],
                                 mybir.ActivationFunctionType.Exp)

            # 3 DMA queues: sync(SP), scalar(ACT), gpsimd. FIFO per queue,
            # ordered by first use: QK0/QK1 need xt+wqk, then V needs w1v0.
            bqk = const_p.tile([128, 2 * ND], F32)
            msk_sb = const_p.tile([128, NT_S], F32)
            bv = const_p.tile([128, D], BF16)
            bo = const_p.tile([128, D], F32)
            nc.gpsimd.dma_start(bqk[:], bqk_d[...])
            nc.gpsimd.dma_start(msk_sb[:], msk_d[...])
            nc.sync.dma_start(xt_sb[:, 0:4, :], xt_lo[...])
            nc.scalar.dma_start(wqks[0][:], wqk_pk[0])
            nc.gpsimd.dma_start(xt_sb[:, 4:8, :], xt_hi[...])
            nc.sync.dma_start(wqks[1][:], wqk_pk[1])
            nc.sync.dma_start(wqks[2][:], wqk_pk[2])
            nc.sync.dma_start(w1v0[:, 0:4, :], w1v0_d[:, 0:4, :])
            nc.scalar.dma_start(w1v0[:, 4:8, :], w1v0_d[:, 4:8, :])
            nc.gpsimd.dma_start(bv[:], bv_d[...])

            for i in (4, 6):
                nc.sync.dma_start(wqks[i][:], wqk_pk[i])
            for i in (3, 5, 7):
                nc.gpsimd.dma_start(wqks[i][:], wqk_pk[i])
            nc.scalar.dma_start(w1v1[:], w1v1_d[...])
            nc.scalar.dma_start(w2sb[:], w2_d[...])
            nc.scalar.dma_start(bo[:], bo_d[...])

            # ones column block for the denominator matmuls
            ones64 = const_p.tile([128, 64], BF16)
            nc.gpsimd.memset(ones64[:], 1.0)
            # keep[k, q] = 1 if q >= k else 0, replicated for both heads
            tril2 = const_p.tile([128, 2, 128], BF16)
            make_upper_triangular(nc, tril2[:, 0, :], val=1.0, diag=True)
            make_upper_triangular(nc, tril2[:, 1, :], val=1.0, diag=True)
            pad_bias = const_p.tile([128, NT_S], F32)
            nc.vector.tensor_scalar(
                out=pad_bias[:], in0=msk_sb[:], scalar1=1.0, scalar2=1e9,
                op0=mybir.AluOpType.subtract, op1=mybir.AluOpType.mult,
            )

            # ---------------- emit helpers ----------------
            def emit_qk_pass(i):
                psq = mmps_p.tile([128, 512], F32, tag="mmps")
                psk = mmps_p.tile([128, 512], F32, tag="mmps")
                for d in range(ND):
                    nc.tensor.matmul(psq[:], wqks[i][:, d, 0, :], xt_sb[:, d, :],
                                     start=(d == 0), stop=(d == ND - 1))
                    nc.tensor.matmul(psk[:], wqks[i][:, d, 1, :], xt_sb[:, d, :],
                                     start=(d == 0), stop=(d == ND - 1))
                qt = qkt_p.tile([128, S], BF16, tag="qkt")
                kt = qkt_p.tile([128, S], BF16, tag="qkt")
                nc.scalar.activation(qt[:], psq[:], Ident,
                                     bias=bqk[:, i : i + 1], scale=1.0)
                nc.scalar.activation(kt[:], psk[:], Ident,
                                     bias=bqk[:, ND + i : ND + i + 1], scale=1.0)
                return qt, kt

            def emit_v(c, w1v_c):
                # V projection c-half: heads 8c..8c+7 -> va cols c*512..+512
                for t in range(NT_S):
                    ps = mmps_p.tile([128, 512], F32, tag="mmps")
                    for d in range(ND):
                        nc.tensor.matmul(ps[:], xt_sb[:, d, t * 128 : (t + 1) * 128],
                                         w1v_c[:, d, :],
                                         start=(d == 0), stop=(d == ND - 1))
                    # (gpsimd cannot read PSUM -> all psum-draining ops on DVE)
                    nc.vector.tensor_tensor(
                        out=va_sb[:, t, c * 512 : (c + 1) * 512], in0=ps[:],
                        in1=bv[:, c * 512 : (c + 1) * 512], op=ADD)

            def emit_scores(i, qt, kt):
                pts = []
                for sk in range(NT_S):
                    w = S - sk * 128
                    sc = scps_p.tile([128, 2, 512], F32, tag="scps")
                    nc.tensor.matmul(sc[:, 0, 0:w], kt[0:64, sk * 128 : (sk + 1) * 128],
                                     qt[0:64, sk * 128 : S], start=True, stop=True)
                    nc.tensor.matmul(sc[:, 1, 0:w], kt[64:128, sk * 128 : (sk + 1) * 128],
                                     qt[64:128, sk * 128 : S], start=True, stop=True)
                    pt = pt_p.tile([128, 2, 512], BF16, tag="pt")
                    nc.scalar.activation(pt[:, :, 0:w], sc[:, :, 0:w], Exp,
                                         bias=pad_bias[:, sk : sk + 1], scale=SCALE)
                    # zero strictly-future entries of the diagonal block
                    nc.vector.tensor_tensor(out=pt[:, :, 0:128], in0=pt[:, :, 0:128],
                                            in1=tril2[:], op=MUL)
                    pts.append(pt)
                return pts

            def emit_av(i, pts):
                h_e, h_o = 2 * i, 2 * i + 1
                # packed denominators: den_e on partitions 0:64, den_o on
                # 64:128 of ONE tile -> a single unshifted reciprocal
                dn = dnps_p.tile([128, 512], F32, tag="dnps")
                av2 = avps_p.tile([128, 512], F32, tag="avps")
                for sk in range(NT_S):
                    w = S - sk * 128
                    nc.tensor.matmul(dn[0:64, sk * 128 : S], ones64[:],
                                     pts[sk][:, 0, 0:w],
                                     start=(sk == 0), stop=(sk == NT_S - 1))
                    nc.tensor.matmul(dn[64:128, sk * 128 : S], ones64[:],
                                     pts[sk][:, 1, 0:w],
                                     start=(sk == 0), stop=(sk == NT_S - 1))
                # chunked recip+normalize for the last pairs: c_proj reads
                # at[:, i, :] in 128-col slices, so let them unblock
                # progressively (these norms sit in the c_proj critical path)
                nch = 4 if i >= ND - 2 else 1
                cw = 512 // nch
                rc = rc_p.tile([128, 512], F32, tag="rc")
                for ch in range(nch):
                    nc.vector.reciprocal(out=rc[:, ch * cw : (ch + 1) * cw],
                                         in_=dn[:, ch * cw : (ch + 1) * cw])
                for sk in range(NT_S):
                    w = S - sk * 128
                    nc.tensor.matmul(av2[0:64, sk * 128 : S],
                                     va_sb[:, sk, h_e * 64 : h_e * 64 + 64],
                                     pts[sk][:, 0, 0:w],
                                     start=(sk == 0), stop=(sk == NT_S - 1))
                    nc.tensor.matmul(av2[64:128, sk * 128 : S],
                                     va_sb[:, sk, h_o * 64 : h_o * 64 + 64],
                                     pts[sk][:, 1, 0:w],
                                     start=(sk == 0), stop=(sk == NT_S - 1))
                for ch in range(nch):
                    sl = slice(ch * cw, (ch + 1) * cw)
                    nc.vector.tensor_tensor(out=at_sb[:, i, sl],
                                            in0=av2[:, sl], in1=rc[:, sl],
                                            op=MUL)

            # ---------------- schedule ----------------
            # Front-load QK passes 0-2 + scores(0) so the PE has work while
            # the V weights stream in; V_c0 right before the pair loop.
            qt0, kt0 = emit_qk_pass(0)
            qts = {1: emit_qk_pass(1)}
            if dbg:
                nc.sync.dma_start(dbg_d["dqt"][...], qt0[:])
                nc.sync.dma_start(dbg_d["dkt"][...], kt0[:])
            pts0 = emit_scores(0, qt0, kt0)
            if dbg:
                for sk in range(NT_S):
                    nc.sync.dma_start(dbg_d["dpt"][sk], pts0[sk][:])
            qts[2] = emit_qk_pass(2)
            emit_v(0, w1v0)
            next_qk = 3
            for i in range(ND):
                pts = pts0 if i == 0 else emit_scores(i, *qts.pop(i))
                # independent PE work between scores and A^T hides the exps;
                # V_c1 at i=3: late enough for its DMA, before pair 4 needs it
                if i == 3:
                    emit_v(1, w1v1)
                elif i >= 1 and next_qk < ND:
                    qts[next_qk] = emit_qk_pass(next_qk)
                    next_qk += 1
                elif i == ND - 1:
                    # hide the last pair's exps + reciprocal: run the first
                    # two c_proj chains' d=0..6 steps now, d=7 after norm(7)
                    cpre = []
                    for c in range(2):
                        ps = mmps_p.tile([128, 512], F32, tag="mmps")
                        for d in range(ND - 1):
                            nc.tensor.matmul(ps[:], at_sb[:, d, 0:128],
                                             w2sb[:, d, c * 512 : (c + 1) * 512],
                                             start=(d == 0), stop=False)
                        cpre.append(ps)
                emit_av(i, pts)

            if dbg:
                nc.sync.dma_start(dbg_d["dva"][...], va_sb[:])
                nc.sync.dma_start(dbg_d["dat"][...], at_sb[:])

            # ---------------- c_proj ----------------
            for t in range(NT_S):
                for c in range(2):
                    if t == 0:
                        ps = cpre[c]
                        nc.tensor.matmul(ps[:], at_sb[:, ND - 1, 0:128],
                                         w2sb[:, ND - 1, c * 512 : (c + 1) * 512],
                                         start=False, stop=True)
                    else:
                        ps = mmps_p.tile([128, 512], F32, tag="mmps")
                        for d in range(ND):
                            nc.tensor.matmul(ps[:],
                                             at_sb[:, d, t * 128 : (t + 1) * 128],
                                             w2sb[:, d, c * 512 : (c + 1) * 512],
                                             start=(d == 0), stop=(d == ND - 1))
                    ob = out_p.tile([128, 512], F32, tag="outsb")
                    # final tile: chunk bias+store so the last DMA is small
                    # and the drain tail shrinks
                    nst = 2 if (t == NT_S - 1 and c == 1) else 1
                    sw = 512 // nst
                    for st in range(nst):
                        sl = slice(st * sw, (st + 1) * sw)
                        nc.vector.tensor_tensor(
                            out=ob[:, sl], in0=ps[:, sl],
                            in1=bo[:, c * 512 + st * sw : c * 512 + (st + 1) * sw],
                            op=ADD)
                        # sync engine is idle at the tail; ACT would delay
                        # these issues behind the queued exps
                        nc.sync.dma_start(
                            out[t * 128 : (t + 1) * 128,
                                c * 512 + st * sw : c * 512 + (st + 1) * sw],
                            ob[:, sl])

    nc.compile()
    return nc


def _get_nc(dbg=False):
    key = ("nc", dbg)
    if key not in _CACHED:
        _CACHED[key] = _build_nc(dbg)
    return _CACHED[key]


def _pack_weights(w1, w2):
    """Host-side bf16 packing into [128, ...] contiguous DMA blocks."""
    w1r = w1.reshape(ND, 128, 3 * D)
    qs = w1r[:, :, :D].reshape(ND, 128, ND, 128)       # [d, p, i, c]
    ks = w1r[:, :, D : 2 * D].reshape(ND, 128, ND, 128)
    pk = np.stack([qs, ks], axis=3)                    # [d, p, i, {q,k}, c]
    wqk_pk = pk.transpose(2, 1, 0, 3, 4).astype(NPBF16)  # [i, p, d, {q,k}, c]
    w1v = w1r[:, :, 2 * D :]                           # [d, p, 1024]
    w1v0 = w1v[:, :, :512].transpose(1, 0, 2).astype(NPBF16)   # [p, d, 512]
    w1v1 = w1v[:, :, 512:].transpose(1, 0, 2).astype(NPBF16)
    w2pk = w2.reshape(ND, 128, D).transpose(1, 0, 2).astype(NPBF16)  # [p, d, e]
    return (np.ascontiguousarray(wqk_pk), np.ascontiguousarray(w1v0),
            np.ascontiguousarray(w1v1), np.ascontiguousarray(w2pk))


def _make_in_maps(x, mask, w1, b1, w2, b2):
    wqk_pk, w1v0, w1v1, w2pk = _pack_weights(w1, w2)
    bqk_pk = np.ascontiguousarray(b1[: 2 * D].reshape(2 * ND, 128).T)
    bv_pk = np.ascontiguousarray(
        np.broadcast_to(b1[2 * D :], (128, D)).astype(NPBF16))
    bo_pk = np.ascontiguousarray(np.broadcast_to(b2, (128, D)))
    in_maps = []
    for b in range(N_CORES):
        xtp = x[b].T.reshape(ND, 128, S).transpose(1, 0, 2)  # [p, d, s]
        in_maps.append({
            "xt_lo": np.ascontiguousarray(xtp[:, 0:4].astype(NPBF16)),
            "xt_hi": np.ascontiguousarray(xtp[:, 4:8].astype(NPBF16)),
            "w1v0": w1v0, "w1v1": w1v1, "wqk_pk": wqk_pk, "w2pk": w2pk,
            "mskpk": np.ascontiguousarray(mask[b].reshape(NT_S, 128).T),
            "bqkpk": bqk_pk, "bvpk": bv_pk, "bopk": bo_pk,
        })
    return in_maps


def _trace_setup(inputs):
    """Build (nc, in_maps) exactly as kernel() would — for test.py tracing."""
    x = np.asarray(inputs["x"], dtype=np.float32)
    mask = np.asarray(inputs["mask"], dtype=np.float32)
    w1 = np.ascontiguousarray(np.asarray(inputs["c_attn_w"], dtype=np.float32))
    b1 = np.ascontiguousarray(np.asarray(inputs["c_attn_b"], dtype=np.float32))
    w2 = np.ascontiguousarray(np.asarray(inputs["c_proj_w"], dtype=np.float32))
    b2 = np.ascontiguousarray(np.asarray(inputs["c_proj_b"], dtype=np.float32))
    return _get_nc(), _make_in_maps(x, mask, w1, b1, w2, b2)


def kernel(x, mask, c_attn_w, c_attn_b, c_proj_w, c_proj_b):
    x = np.asarray(x, dtype=np.float32)
    mask = np.asarray(mask, dtype=np.float32)
    w1 = np.ascontiguousarray(np.asarray(c_attn_w, dtype=np.float32))
    b1 = np.ascontiguousarray(np.asarray(c_attn_b, dtype=np.float32))
    w2 = np.ascontiguousarray(np.asarray(c_proj_w, dtype=np.float32))
    b2 = np.ascontiguousarray(np.asarray(c_proj_b, dtype=np.float32))

    nc = _get_nc()
    in_maps = _make_in_maps(x, mask, w1, b1, w2, b2)
    res = run_bass_kernel_spmd(nc, in_maps, list(range(N_CORES)))
    return np.stack([res.results[b]["out"] for b in range(N_CORES)], axis=0)
